# revision 18
# baseline (speedup 1.0000x reference)
"""Trainium2 Bass kernel for nn_ASSC_66657892434080 (v3).

Reference computation (per batch sample b, data-parallel over 8 cores):
    q = wq @ x_1[b] + bq ; k = wk @ x[b] + bk          (1x1 convs)
    proj_query = PSP(q) [256,280] ; proj_key = PSP(k) [32,280]
    aff = sigmoid(proj_query @ proj_key^T)             [256,32]
    out[b] = conv3x3(x_1[b], (aff @ con.reshape(32,-1)))   (grouped conv)

v3 key mechanisms (see kernel_v1/v2 baks for history):
  * contiguous [128, 9216] x loads split over BOTH HWDGE queues (SP + Act).
  * conv borders via partial-extent matmuls; dy shift via 3 SBUF->SBUF DMAs.
  * prefix scans (DVE-only) write TRANSPOSED dsts via hand-built 3-dim APs
    (validated bit-exact on HW): F1T[w,h] so every bin-diff has a contiguous
    inner dim -> diffs run fast on GPSIMD; same trick for the H-stage (F2T).
  * area-normalization folded into scalar_tensor_tensor bias-adds.
  * affinity matmul chain in bf16.
"""

import numpy as np
import dataclasses
import concourse.bass as bass
import concourse.bacc as bacc
import concourse.tile as tile
import concourse.mybir as mybir
import bass_rust
from concourse.bass_utils import run_bass_kernel_spmd

B, C, H, W = 8, 256, 96, 96
C8 = 32
HW = H * W                      # 9216
POOL_SIZES = (1, 3, 5, 7, 14)   # -> 30 1-D bins, 280 2-D positions
NB = sum(POOL_SIZES)            # 30
NP = sum(s * s for s in POOL_SIZES)  # 280
TH = 32                         # k-side rows per third
ROWS = 4                        # conv rows per PSUM chunk
NCH = H // ROWS                 # 24 chunks
HHW = HW // 2                   # 4608
SR = 48                         # strip rows (q side)
F32 = mybir.dt.float32
BF16 = mybir.dt.bfloat16
F16 = mybir.dt.float16

# smalls packing (f32, [128, 696]): bq | bk | ai | ai3(3x32) | id32
SM_BQ, SM_BK, SM_AI, SM_AI3, SM_ID, SM_N = 0, 256, 288, 568, 664, 696


def _pool_bins(n, s):
    return [((i * n) // s, -((-(i + 1) * n) // s)) for i in range(s)]


HBINS = {s: _pool_bins(H, s) for s in POOL_SIZES}
JBASE = {}
B280 = {}
_j = _p = 0
for _s in POOL_SIZES:
    JBASE[_s] = _j
    B280[_s] = _p
    _j += _s
    _p += _s * _s


def _area_inv():
    ai = np.zeros(NP, np.float32)
    for s in POOL_SIZES:
        hb, wb = _pool_bins(H, s), _pool_bins(W, s)
        for o, (hs, he) in enumerate(hb):
            for p, (ws, we) in enumerate(wb):
                ai[B280[s] + o * s + p] = 1.0 / ((he - hs) * (we - ws))
    return ai


def _split_multiwait_ctrl(nc, default_limit=1):
    """walrus in this container rejects instructions carrying more than one
    sem wait; move extras onto preceding same-engine drains.  NEVER split PE
    instructions (reorder window pulls LDWEIGHTS ahead)."""
    for f in nc.m.functions:
        for bb in f.blocks:
            new_list = []
            for inst in bb.instructions:
                si = inst.sync_info
                waits = list(si.on_wait) if si and si.on_wait else []
                mw = default_limit
                if getattr(inst, "engine", None) == mybir.EngineType.PE:
                    mw = 99
                if len(waits) > mw:
                    for k, w in enumerate(waits[:-mw]):
                        pre = mybir.InstDrain(name=f"{inst.name}-w{k}", ins=[], outs=[])
                        pre.engine = inst.engine
                        pre.sync_info = bass_rust.SyncInfo(on_wait=[w], on_update=[])
                        new_list.append(pre)
                    inst.sync_info = bass_rust.SyncInfo(
                        on_wait=waits[-mw:],
                        on_update=list(si.on_update) if si.on_update else [],
                    )
                new_list.append(inst)
            bb.instructions[:] = new_list


def _sv(ap2d, dims):
    """Strided view: keep partition dim, replace free dims with (step, count)."""
    return dataclasses.replace(ap2d, ap=[list(ap2d.ap[0])] + [[s, c] for s, c in dims])


def _uniform_runs(bins):
    """Group consecutive bins into runs with constant boundary strides."""
    runs = []
    i = 0
    n = len(bins)
    while i < n:
        if i == n - 1:
            runs.append((i, 1, 0, 0))
            i += 1
            continue
        ds = bins[i + 1][0] - bins[i][0]
        de = bins[i + 1][1] - bins[i][1]
        j = i + 1
        while (j + 1 < n and bins[j + 1][0] - bins[j][0] == ds
               and bins[j + 1][1] - bins[j][1] == de):
            j += 1
        runs.append((i, j - i + 1, ds, de))
        i = j + 1
    return runs


def _fix_ldweights_waits(nc):
    """Move waits that gate weight data from InstMatmult to its InstLdweights
    (prevents stale-weight races after Tile's 2-byte matmul split)."""
    import copy
    for f in nc.m.functions:
        for bb in f.blocks:
            insts = bb.instructions
            new_list = []
            i = 0
            while i < len(insts):
                inst = insts[i]
                nxt = insts[i + 1] if i + 1 < len(insts) else None
                if (type(inst).__name__ == "InstLdweights" and nxt is not None
                        and type(nxt).__name__ == "InstMatmult"):
                    wl = list(inst.sync_info.on_wait) if inst.sync_info and inst.sync_info.on_wait else []
                    wm = list(nxt.sync_info.on_wait) if nxt.sync_info and nxt.sync_info.on_wait else []
                    waits = wl + wm
                    mm_upd = list(nxt.sync_info.on_update) if nxt.sync_info and nxt.sync_info.on_update else []
                    ld_upd = list(inst.sync_info.on_update) if inst.sync_info and inst.sync_info.on_update else []
                    if len(waits) > 1:
                        for k, w in enumerate(waits[:-1]):
                            pre = copy.deepcopy(inst)
                            pre.name = f"{inst.name}-ldw{k}"
                            pre.sync_info = bass_rust.SyncInfo(on_wait=[w], on_update=[])
                            new_list.append(pre)
                        inst.sync_info = bass_rust.SyncInfo(on_wait=[waits[-1]], on_update=ld_upd)
                        nxt.sync_info = bass_rust.SyncInfo(on_wait=[], on_update=mm_upd)
                    elif len(waits) == 1:
                        inst.sync_info = bass_rust.SyncInfo(on_wait=[waits[0]], on_update=ld_upd)
                        nxt.sync_info = bass_rust.SyncInfo(on_wait=[], on_update=mm_upd)
                    new_list.append(inst)
                    new_list.append(nxt)
                    i += 2
                    continue
                new_list.append(inst)
                i += 1
            bb.instructions[:] = new_list


def _scan3(nc, dst_slice, dst_dims, src, zcol, n):
    """tensor_tensor_scan with a hand-built multi-dim (e.g. transposed) dst AP.
    Bypasses the 2-dim assert in bass; validated bit-exact on HW.  `zcol` is a
    [*, 1] column whose (bypassed) read also serves as an ordering token."""
    eng = nc.vector
    add, byp = mybir.AluOpType.add, mybir.AluOpType.bypass
    dst3 = dataclasses.replace(
        dst_slice, ap=[list(dst_slice.ap[0])] + [list(d) for d in dst_dims])
    return eng.add_instruction(
        mybir.InstTensorScalarPtr(
            name=eng.bass.get_next_instruction_name(),
            is_tensor_tensor_scan=True,
            is_scalar_tensor_tensor=True,
            op0=add, op1=byp,
            ins=[eng.lower_ap(src),
                 eng.lower_ap_or_imm(0.0),
                 eng.lower_ap(_sv(zcol, [(0, n)]))],
            outs=[eng.lower_ap(dst3)],
        ))


def build_kernel(split_ctrl=True, debug_taps=False):
    nc = bacc.Bacc("TRN2", target_bir_lowering=False, debug=False)
    add, byp = mybir.AluOpType.add, mybir.AluOpType.bypass
    sub, mul = mybir.AluOpType.subtract, mybir.AluOpType.mult

    x1 = nc.dram_tensor("x1", [2, 128, HW], BF16, kind="ExternalInput")
    xx = nc.dram_tensor("xx", [2, 128, HW], BF16, kind="ExternalInput")
    wqT = nc.dram_tensor("wqT", [2, 128, C], BF16, kind="ExternalInput")
    wkTb = nc.dram_tensor("wkTb", [2, 128, C8], BF16, kind="ExternalInput")
    conz = nc.dram_tensor("conz", [6, 128, 96], BF16, kind="ExternalInput")
    smalls = nc.dram_tensor("smalls", [128, SM_N], F32, kind="ExternalInput")
    out = nc.dram_tensor("out", [2, 128, HW], BF16, kind="ExternalOutput")
    if debug_taps:
        dPq0 = nc.dram_tensor("dPq0", [128, NP], F32, kind="ExternalOutput")
        dPq1 = nc.dram_tensor("dPq1", [128, NP], F32, kind="ExternalOutput")
        dDk = nc.dram_tensor("dDk", [96, NP], F32, kind="ExternalOutput")
        dAff = nc.dram_tensor("dAff", [96, C], F32, kind="ExternalOutput")
        dKq = nc.dram_tensor("dKq", [96, TH * W], F32, kind="ExternalOutput")
        dG0 = nc.dram_tensor("dG0", [128, NB * H], F32, kind="ExternalOutput")

    with tile.TileContext(nc) as tc:
        with (
            tc.tile_pool(name="consts", bufs=1) as cpool,
            tc.tile_pool(name="xpool", bufs=1) as xpool,
            tc.tile_pool(name="scratch", bufs=1) as spool,
            tc.tile_pool(name="ostage", bufs=2) as opool,
        ):
            # ---- tiles ----
            czt = cpool.tile([128, 576], BF16, tag="czt", name="czt")
            wkt = cpool.tile([128, 2 * C8], BF16, tag="wkt", name="wkt")
            wqt = cpool.tile([128, 2 * C], BF16, tag="wqt", name="wqt")
            smt = cpool.tile([128, SM_N], F32, tag="smt", name="smt")
            x1t = [xpool.tile([128, HW], BF16, tag=f"x1t{i}", name=f"x1t{i}") for i in range(2)]
            xxt = [xpool.tile([128, HW], BF16, tag=f"xxt{i}", name=f"xxt{i}") for i in range(2)]

            F1s = [spool.tile([128, HHW + 1], F16, tag=f"F1{i}", name=f"F1{i}") for i in range(3)]
            Gq = [spool.tile([128, NB * H], F16, tag=f"G{i}", name=f"G{i}") for i in range(2)]
            F2q = [spool.tile([128, NB * H + 1], F16, tag=f"F2{i}", name=f"F2{i}") for i in range(2)]
            F1k = spool.tile([96, TH * W + 1], F16, tag="F1k", name="F1k")
            Gk = spool.tile([96, NB * TH], F16, tag="Gk", name="Gk")
            F2k = spool.tile([96, NB * TH + 1], F16, tag="F2k", name="F2k")
            kq = spool.tile([96, TH * W], F16, tag="kq", name="kq")
            zs_raw = spool.tile([96, HW], BF16, tag="zsr", name="zsr")
            zss = spool.tile([96, HW], BF16, tag="zss", name="zss")
            Pq = [spool.tile([128, NP], BF16, tag=f"Pq{i}", name=f"Pq{i}") for i in range(2)]
            Dk = spool.tile([96, NP], F32, tag="Dk", name="Dk")
            Dsh = [spool.tile([32, NP], F32, tag=f"Dsh{t}", name=f"Dsh{t}") for t in range(2)]

            # ---- input DMAs: x1 on SP queue (pooling-critical), xx on Act ----
            Q4 = HHW // 2
            nc.sync.dma_start(x1t[0][:, :Q4], x1.ap()[0][:, :Q4])
            nc.sync.dma_start(x1t[0][:, Q4:HHW], x1.ap()[0][:, Q4:HHW])
            nc.sync.dma_start(x1t[0][:, HHW:], x1.ap()[0][:, HHW:])
            nc.sync.dma_start(x1t[1][:, :HHW], x1.ap()[1][:, :HHW])
            nc.sync.dma_start(x1t[1][:, HHW:], x1.ap()[1][:, HHW:])
            nc.sync.dma_start(czt[:], _sv(conz.ap()[0], [(128 * 96, 6), (1, 96)]))
            nc.sync.dma_start(wkt[:], _sv(wkTb.ap()[0], [(128 * C8, 2), (1, C8)]))
            nc.scalar.dma_start(smt[:], smalls.ap())
            nc.scalar.dma_start(wqt[:], _sv(wqT.ap()[0], [(128 * C, 2), (1, C)]))
            nc.scalar.dma_start(xxt[0][:], xx.ap()[0])
            nc.scalar.dma_start(xxt[1][:], xx.ap()[1])

            bq_t = smt[:, SM_BQ:SM_BQ + C]
            bk_t = smt[:, SM_BK:SM_BK + C8]
            id32 = smt[:, SM_ID:SM_ID + 32]

            # ---- DVE warmup + zero-region init ----
            for t_ in F1s:
                nc.vector.memset(t_[:, 0:1], 0.0)
            nc.vector.memset(F1k[:96, 0:1], 0.0)
            for t_ in F2q:
                nc.vector.memset(t_[:, 0:1], 0.0)
            nc.vector.memset(F2k[:96, 0:1], 0.0)
            nc.gpsimd.memset(Dk[:, :], 0.0)
            nc.gpsimd.memset(zss[0:32, 0:W], 0.0)
            nc.gpsimd.memset(zss[64:96, (H - 1) * W:HW], 0.0)

            # ---- zconv: border-split partial matmuls ----
            def zconv_chunk(pzp, j):
                r0 = j * ROWS
                zp = pzp.tile([96, ROWS * W], F32, tag="zp", name="zp")
                base = r0 * W
                for cinc in range(2):   # dx=1 center, full width, start
                    nc.tensor.matmul(
                        zp[:, :], czt[:, (2 + cinc) * 96:(3 + cinc) * 96],
                        _sv(x1t[cinc][:, base:], [(W, ROWS), (1, W)]),
                        start=(cinc == 0), stop=False)
                for cinc in range(2):   # dx=0 left tap: out 1..95 <- x 0..94
                    nc.tensor.matmul(
                        _sv(zp[:, 1:], [(W, ROWS), (1, W - 1)]),
                        czt[:, (0 + cinc) * 96:(1 + cinc) * 96],
                        _sv(x1t[cinc][:, base:], [(W, ROWS), (1, W - 1)]),
                        start=False, stop=False)
                for cinc in range(2):   # dx=2 right tap: out 0..94 <- x 1..95
                    nc.tensor.matmul(
                        _sv(zp[:, 0:], [(W, ROWS), (1, W - 1)]),
                        czt[:, (4 + cinc) * 96:(5 + cinc) * 96],
                        _sv(x1t[cinc][:, base + 1:], [(W, ROWS), (1, W - 1)]),
                        start=False, stop=(cinc == 1))
                nc.scalar.copy(zs_raw[:, base:base + ROWS * W], zp[:, :])

            with tc.tile_pool(name="pz", bufs=3, space="PSUM") as pzp, \
                 tc.tile_pool(name="pk", bufs=2, space="PSUM") as pkp:
                for j in range(12):
                    zconv_chunk(pzp, j)
                # k-proj off-major: one [96,512] copy per off
                for off in range(6):
                    kp = pkp.tile([128, 512], F32, tag="kp", name="kp")
                    for t in range(3):
                        for cc in range(2):
                            nc.tensor.matmul(kp[32 * t:32 * t + 32, :],
                                             wkt[:, cc * C8:(cc + 1) * C8],
                                             xxt[cc][:, (t * 6 + off) * 512:(t * 6 + off + 1) * 512],
                                             start=(cc == 0), stop=(cc == 1),
                                             tile_position=(0, 32 * t))
                    nc.scalar.copy(kq[0:96, off * 512:(off + 1) * 512], kp[0:96, :])
                for j in range(12, NCH):
                    zconv_chunk(pzp, j)

            # ---- pooling: h-major prefix scans (DVE), diffs split DVE/GPSIMD ----
            add_, byp_ = mybir.AluOpType.add, mybir.AluOpType.bypass

            def scan(F1, src, n):
                nc.vector.tensor_tensor_scan(
                    F1[:, 1:1 + n], src, _sv(F1[:, 0:1], [(0, n)]), 0.0, add_, byp_)

            def w_diffs(eng, F1, G, h0, rows, part=128):
                jbase = 0
                for s_ in POOL_SIZES:
                    sb = _pool_bins(W, s_)
                    for (i0_, cnt, ds, de) in _uniform_runs(sb):
                        ws, we = sb[i0_]
                        j0 = jbase + i0_
                        eng.tensor_tensor(
                            _sv(G[:part, j0 * H + h0:], [(H, cnt), (1, rows)]) if part == 128 else
                            _sv(G[:part, j0 * rows + h0:], [(rows, cnt), (1, rows)]),
                            _sv(F1[:part, we:], [(de, cnt), (W, rows)]),
                            _sv(F1[:part, ws:], [(ds, cnt), (W, rows)]),
                            mybir.AluOpType.subtract)
                    jbase += s_

            def h_diffs(eng, F2, P_out):
                for s_ in POOL_SIZES:
                    jb = JBASE[s_]
                    for (o0, cnt, ds, de) in _uniform_runs(HBINS[s_]):
                        hs, he = HBINS[s_][o0]
                        eng.tensor_tensor(
                            _sv(P_out[:, B280[s_] + o0 * s_:], [(s_, cnt), (1, s_)]),
                            _sv(F2[:, jb * H + he:], [(de, cnt), (H, s_)]),
                            _sv(F2[:, jb * H + hs:], [(ds, cnt), (H, s_)]),
                            mybir.AluOpType.subtract)

            # q scans: strips 0..3 = (cc, strip); F1 buffers rotate a,b,c,a
            FB = [F1s[0], F1s[1], F1s[2], F1s[0]]
            for idx, (cc, strip) in enumerate([(0, 0), (0, 1), (1, 0), (1, 1)]):
                scan(FB[idx], x1t[cc][:, strip * HHW:(strip + 1) * HHW], HHW)
                if idx == 0:  # strip-0 W-diffs on GPSIMD (overlap later scans)
                    w_diffs(nc.gpsimd, FB[idx], Gq[cc], strip * SR, SR)
            # strip-1 diffs on DVE (GPSIMD too slow to keep up), then kW scan
            w_diffs(nc.vector, FB[1], Gq[0], SR, SR)
            scan(F1k, kq[:, :], TH * W)
            # k W-diffs on GPSIMD
            jbase = 0
            for s_ in POOL_SIZES:
                sb = _pool_bins(W, s_)
                for (i0_, cnt, ds, de) in _uniform_runs(sb):
                    ws, we = sb[i0_]
                    j0 = jbase + i0_
                    nc.gpsimd.tensor_tensor(
                        _sv(Gk[:96, j0 * TH:], [(TH, cnt), (1, TH)]),
                        _sv(F1k[:96, we:], [(de, cnt), (W, TH)]),
                        _sv(F1k[:96, ws:], [(ds, cnt), (W, TH)]),
                        mybir.AluOpType.subtract)
                jbase += s_
            scan(F2q[0], Gq[0][:, :], NB * H)
            scan(F2k, Gk[:96, :], NB * TH)
            # k H-diffs: third-local clipped bins, one instr per bin (GPSIMD)
            for t in range(3):
                for s_ in POOL_SIZES:
                    jb = JBASE[s_]
                    for o, (hs, he) in enumerate(HBINS[s_]):
                        lhs = min(max(hs - TH * t, 0), TH)
                        lhe = min(max(he - TH * t, 0), TH)
                        if lhe <= lhs:
                            continue
                        nc.gpsimd.tensor_tensor(
                            _sv(Dk[32 * t:32 * t + 32, B280[s_] + o * s_:], [(1, s_)]),
                            _sv(F2k[32 * t:32 * t + 32, jb * TH + lhe:], [(TH, s_)]),
                            _sv(F2k[32 * t:32 * t + 32, jb * TH + lhs:], [(TH, s_)]),
                            mybir.AluOpType.subtract)
            # chunk-1 W-diffs (DVE), q1 F2, H-diffs (DVE)
            w_diffs(nc.vector, FB[2], Gq[1], 0, SR)
            w_diffs(nc.vector, FB[3], Gq[1], SR, SR)
            scan(F2q[1], Gq[1][:, :], NB * H)
            h_diffs(nc.vector, F2q[0], Pq[0])
            h_diffs(nc.vector, F2q[1], Pq[1])

            # partition-shift thirds 1,2 of Dk down to base 0 for matmuls
            # (on SP queue, after zss shifts below in emission but ready later)
            nc.sync.dma_start(zss[0:32, W:HW], zs_raw[0:32, 0:HW - W])
            nc.sync.dma_start(zss[32:64, :], zs_raw[32:64, :])
            nc.sync.dma_start(zss[64:96, 0:HW - W], zs_raw[64:96, W:HW])
            for t in range(2):
                nc.sync.dma_start(Dsh[t][:, :], Dk[32 * (t + 1):32 * (t + 2), :])

            # ---- projections / affinity (bf16 chain) ----
            PCH = [(0, 128), (128, 128), (256, 24)]
            pqT = [cpool.tile([n, C], BF16, tag=f"pqT{i}", name=f"pqT{i}")
                   for i, (_, n) in enumerate(PCH)]
            pkR = [cpool.tile([n, 96], BF16, tag=f"pkR{i}", name=f"pkR{i}")
                   for i, (_, n) in enumerate(PCH)]
            affT96 = cpool.tile([96, C], BF16, tag="affT96", name="affT96")

            with tc.tile_pool(name="psmall", bufs=2, space="PSUM") as pps:
                for i, (p0, n) in enumerate(PCH):
                    aicol = smt[:n, SM_AI3 + 32 * i:SM_AI3 + 32 * i + 1]
                    ps2 = pps.tile([n, C8], F32, tag="ps2", name="ps2")
                    dsrc = [Dk, Dsh[0], Dsh[1]]
                    for t in range(3):
                        nc.tensor.matmul(ps2[:], dsrc[t][0:32, p0:p0 + n],
                                         id32[0:32, :],
                                         start=(t == 0), stop=(t == 2))
                    for g in range(3):
                        nc.vector.scalar_tensor_tensor(
                            pkR[i][:, 32 * g:32 * g + 32], ps2[:], aicol,
                            bk_t[:n, :], mul, add)
                for i, (p0, n) in enumerate(PCH):
                    aicol = smt[:n, SM_AI3 + 32 * i:SM_AI3 + 32 * i + 1]
                    ps = pps.tile([n, C], F32, tag="ps", name="ps")
                    for cc in range(2):
                        nc.tensor.matmul(ps[:], Pq[cc][:, p0:p0 + n],
                                         wqt[:, cc * C:(cc + 1) * C],
                                         start=(cc == 0), stop=(cc == 1))
                    nc.vector.scalar_tensor_tensor(
                        pqT[i][:], ps[:], aicol, bq_t[:n, :], mul, add)
                pa = pps.tile([96, C], F32, tag="pa", name="pa")
                for i in range(3):
                    nc.tensor.matmul(pa[:], pkR[i][:], pqT[i][:],
                                     start=(i == 0), stop=(i == 2))
                nc.scalar.activation(affT96[:], pa[:], mybir.ActivationFunctionType.Sigmoid)

            # PE prewarm: ramp tensor clock before the out burst (deps on
            # late tile Pq[1] so the scheduler runs these in the pre-aff gap)
            with tc.tile_pool(name="pw", bufs=1, space="PSUM") as pwp:
                pwt = pwp.tile([96, C], F32, tag="pwt", name="pwt")
                for _r in range(24):
                    nc.tensor.matmul(pwt[:], czt[:, 0:96], Pq[1][:, 0:256],
                                     start=True, stop=True)

            # ---- out matmuls: 24 chunks x 2 cout halves; GB=6 staging ----
            GB = 6
            with tc.tile_pool(name="po", bufs=4, space="PSUM") as pop:
                ot = [None, None]
                for j in range(NCH):
                    r0 = j * ROWS
                    for coutc in range(2):
                        op = pop.tile([128, ROWS * W], F32, tag="op", name="op")
                        nc.tensor.matmul(op[:], affT96[:, coutc * 128:(coutc + 1) * 128],
                                         zss[:, r0 * W:(r0 + ROWS) * W],
                                         start=True, stop=True)
                        if j % GB == 0:
                            ot[coutc] = opool.tile([128, GB * ROWS * W], BF16,
                                                   tag=f"ot{coutc}", name=f"ot{coutc}")
                        seg = (j % GB) * ROWS * W
                        if (j + coutc) % 2 == 0:
                            nc.scalar.copy(ot[coutc][:, seg:seg + ROWS * W], op[:])
                        else:
                            nc.vector.tensor_copy(ot[coutc][:, seg:seg + ROWS * W], op[:])
                        if j % GB == GB - 1:
                            nc.sync.dma_start(
                                out.ap()[coutc][:, (r0 - (GB - 1) * ROWS) * W:(r0 + ROWS) * W],
                                ot[coutc][:])

            if debug_taps:
                dbg = cpool.tile([128, NP], F32, tag="dbg", name="dbg")
                dbg2 = cpool.tile([128, TH * W], F32, tag="dbg2", name="dbg2")
                nc.vector.tensor_copy(dbg[:, :], Pq[0][:, :])
                nc.sync.dma_start(dPq0.ap(), dbg[:, :])
                nc.vector.tensor_copy(dbg[:, :], Pq[1][:, :])
                nc.sync.dma_start(dPq1.ap(), dbg[:, :])
                nc.sync.dma_start(dDk.ap(), Dk[:, :])
                nc.vector.tensor_copy(dbg[0:96, 0:C], affT96[:, :])
                nc.sync.dma_start(dAff.ap(), dbg[0:96, 0:C])
                nc.vector.tensor_copy(dbg2[0:96, 0:TH * W], kq[:, :])
                nc.sync.dma_start(dKq.ap(), dbg2[0:96, 0:TH * W])
                nc.vector.tensor_copy(dbg2[:, 0:NB * H], Gq[0][:, :])
                nc.sync.dma_start(dG0.ap(), dbg2[:, 0:NB * H])

    if split_ctrl:
        nc.compile()
        _fix_ldweights_waits(nc)
    return nc


_NC_CACHE = {}


def _get_nc():
    if "nc" not in _NC_CACHE:
        _NC_CACHE["nc"] = build_kernel()
    return _NC_CACHE["nc"]


def _conv_cast(x):
    import ml_dtypes
    return np.ascontiguousarray(x, np.float32).astype(ml_dtypes.bfloat16)


def kernel(x_1, x, wq, bq, wk, bk, con):
    import ml_dtypes
    x_1 = _conv_cast(x_1)
    x = _conv_cast(x)
    con = np.asarray(con, np.float32)
    wq = np.asarray(wq, np.float32)
    bq = np.asarray(bq, np.float32)
    wk = np.asarray(wk, np.float32)
    bk = np.asarray(bk, np.float32)

    wqT_h = np.ascontiguousarray(wq.T).reshape(2, 128, C).astype(ml_dtypes.bfloat16)
    wkTb_h = np.ascontiguousarray(wk.T).reshape(2, 128, C8).astype(ml_dtypes.bfloat16)
    conz_h = np.ascontiguousarray(
        con.transpose(3, 1, 2, 0)          # [dx, cin256, dy, kk]
        .reshape(3, 2, 128, 3 * C8)
        .reshape(6, 128, 96)
    ).astype(ml_dtypes.bfloat16)
    ai = _area_inv()
    smalls_h = np.zeros((128, SM_N), np.float32)
    smalls_h[:, SM_BQ:SM_BQ + C] = bq[None, :]
    smalls_h[:, SM_BK:SM_BK + C8] = bk[None, :]
    smalls_h[:, SM_AI:SM_AI + NP] = ai[None, :]
    for i, (p0, n) in enumerate([(0, 128), (128, 128), (256, 24)]):
        smalls_h[:n, SM_AI3 + 32 * i:SM_AI3 + 32 * (i + 1)] = ai[p0:p0 + n, None]
    smalls_h[:32, SM_ID:SM_ID + 32] = np.eye(32, dtype=np.float32)

    in_maps = []
    for b in range(B):
        in_maps.append({
            "x1": x_1[b].reshape(2, 128, HW),
            "xx": x[b].reshape(2, 128, HW),
            "wqT": wqT_h, "wkTb": wkTb_h, "conz": conz_h, "smalls": smalls_h,
        })
    global _last_in_maps
    _last_in_maps = in_maps
    nc = _get_nc()
    res = run_bass_kernel_spmd(nc, in_maps, list(range(B)))
    return np.stack([res.results[b]["out"].astype(np.float32).reshape(C, H, W)
                     for b in range(B)])


# revision 19
# speedup vs baseline: 1.0584x; 1.0584x over previous
"""Trainium2 Bass kernel for nn_ASSC_66657892434080 (v3).

Reference computation (per batch sample b, data-parallel over 8 cores):
    q = wq @ x_1[b] + bq ; k = wk @ x[b] + bk          (1x1 convs)
    proj_query = PSP(q) [256,280] ; proj_key = PSP(k) [32,280]
    aff = sigmoid(proj_query @ proj_key^T)             [256,32]
    out[b] = conv3x3(x_1[b], (aff @ con.reshape(32,-1)))   (grouped conv)

v3 key mechanisms (see kernel_v1/v2 baks for history):
  * contiguous [128, 9216] x loads split over BOTH HWDGE queues (SP + Act).
  * conv borders via partial-extent matmuls; dy shift via 3 SBUF->SBUF DMAs.
  * prefix scans (DVE-only) write TRANSPOSED dsts via hand-built 3-dim APs
    (validated bit-exact on HW): F1T[w,h] so every bin-diff has a contiguous
    inner dim -> diffs run fast on GPSIMD; same trick for the H-stage (F2T).
  * area-normalization folded into scalar_tensor_tensor bias-adds.
  * affinity matmul chain in bf16.
"""

import numpy as np
import dataclasses
import concourse.bass as bass
import concourse.bacc as bacc
import concourse.tile as tile
import concourse.mybir as mybir
import bass_rust
from concourse.bass_utils import run_bass_kernel_spmd

B, C, H, W = 8, 256, 96, 96
C8 = 32
HW = H * W                      # 9216
POOL_SIZES = (1, 3, 5, 7, 14)   # -> 30 1-D bins, 280 2-D positions
NB = sum(POOL_SIZES)            # 30
NP = sum(s * s for s in POOL_SIZES)  # 280
TH = 32                         # k-side rows per third
ROWS = 4                        # conv rows per PSUM chunk
NCH = H // ROWS                 # 24 chunks
HHW = HW // 2                   # 4608
SR = 48                         # strip rows (q side)
F32 = mybir.dt.float32
BF16 = mybir.dt.bfloat16
F16 = mybir.dt.float16

# smalls packing (f32, [128, 696]): bq | bk | ai | ai3(3x32) | id32
SM_BQ, SM_BK, SM_AI, SM_AI3, SM_ID, SM_N = 0, 256, 288, 568, 664, 696


def _pool_bins(n, s):
    return [((i * n) // s, -((-(i + 1) * n) // s)) for i in range(s)]


HBINS = {s: _pool_bins(H, s) for s in POOL_SIZES}
JBASE = {}
B280 = {}
_j = _p = 0
for _s in POOL_SIZES:
    JBASE[_s] = _j
    B280[_s] = _p
    _j += _s
    _p += _s * _s


def _area_inv():
    ai = np.zeros(NP, np.float32)
    for s in POOL_SIZES:
        hb, wb = _pool_bins(H, s), _pool_bins(W, s)
        for o, (hs, he) in enumerate(hb):
            for p, (ws, we) in enumerate(wb):
                ai[B280[s] + o * s + p] = 1.0 / ((he - hs) * (we - ws))
    return ai


def _split_multiwait_ctrl(nc, default_limit=1):
    """walrus in this container rejects instructions carrying more than one
    sem wait; move extras onto preceding same-engine drains.  NEVER split PE
    instructions (reorder window pulls LDWEIGHTS ahead)."""
    for f in nc.m.functions:
        for bb in f.blocks:
            new_list = []
            for inst in bb.instructions:
                si = inst.sync_info
                waits = list(si.on_wait) if si and si.on_wait else []
                mw = default_limit
                if getattr(inst, "engine", None) == mybir.EngineType.PE:
                    mw = 99
                if len(waits) > mw:
                    for k, w in enumerate(waits[:-mw]):
                        pre = mybir.InstDrain(name=f"{inst.name}-w{k}", ins=[], outs=[])
                        pre.engine = inst.engine
                        pre.sync_info = bass_rust.SyncInfo(on_wait=[w], on_update=[])
                        new_list.append(pre)
                    inst.sync_info = bass_rust.SyncInfo(
                        on_wait=waits[-mw:],
                        on_update=list(si.on_update) if si.on_update else [],
                    )
                new_list.append(inst)
            bb.instructions[:] = new_list


def _sv(ap2d, dims):
    """Strided view: keep partition dim, replace free dims with (step, count)."""
    return dataclasses.replace(ap2d, ap=[list(ap2d.ap[0])] + [[s, c] for s, c in dims])


def _uniform_runs(bins):
    """Group consecutive bins into runs with constant boundary strides."""
    runs = []
    i = 0
    n = len(bins)
    while i < n:
        if i == n - 1:
            runs.append((i, 1, 0, 0))
            i += 1
            continue
        ds = bins[i + 1][0] - bins[i][0]
        de = bins[i + 1][1] - bins[i][1]
        j = i + 1
        while (j + 1 < n and bins[j + 1][0] - bins[j][0] == ds
               and bins[j + 1][1] - bins[j][1] == de):
            j += 1
        runs.append((i, j - i + 1, ds, de))
        i = j + 1
    return runs


def _fix_ldweights_waits(nc):
    """Move waits that gate weight data from InstMatmult to its InstLdweights
    (prevents stale-weight races after Tile's 2-byte matmul split)."""
    import copy
    for f in nc.m.functions:
        for bb in f.blocks:
            insts = bb.instructions
            new_list = []
            i = 0
            while i < len(insts):
                inst = insts[i]
                nxt = insts[i + 1] if i + 1 < len(insts) else None
                if (type(inst).__name__ == "InstLdweights" and nxt is not None
                        and type(nxt).__name__ == "InstMatmult"):
                    wl = list(inst.sync_info.on_wait) if inst.sync_info and inst.sync_info.on_wait else []
                    wm = list(nxt.sync_info.on_wait) if nxt.sync_info and nxt.sync_info.on_wait else []
                    waits = wl + wm
                    mm_upd = list(nxt.sync_info.on_update) if nxt.sync_info and nxt.sync_info.on_update else []
                    ld_upd = list(inst.sync_info.on_update) if inst.sync_info and inst.sync_info.on_update else []
                    if len(waits) > 1:
                        for k, w in enumerate(waits[:-1]):
                            pre = copy.deepcopy(inst)
                            pre.name = f"{inst.name}-ldw{k}"
                            pre.sync_info = bass_rust.SyncInfo(on_wait=[w], on_update=[])
                            new_list.append(pre)
                        inst.sync_info = bass_rust.SyncInfo(on_wait=[waits[-1]], on_update=ld_upd)
                        nxt.sync_info = bass_rust.SyncInfo(on_wait=[], on_update=mm_upd)
                    elif len(waits) == 1:
                        inst.sync_info = bass_rust.SyncInfo(on_wait=[waits[0]], on_update=ld_upd)
                        nxt.sync_info = bass_rust.SyncInfo(on_wait=[], on_update=mm_upd)
                    new_list.append(inst)
                    new_list.append(nxt)
                    i += 2
                    continue
                new_list.append(inst)
                i += 1
            bb.instructions[:] = new_list


def _scan3(nc, dst_slice, dst_dims, src, zcol, n):
    """tensor_tensor_scan with a hand-built multi-dim (e.g. transposed) dst AP.
    Bypasses the 2-dim assert in bass; validated bit-exact on HW.  `zcol` is a
    [*, 1] column whose (bypassed) read also serves as an ordering token."""
    eng = nc.vector
    add, byp = mybir.AluOpType.add, mybir.AluOpType.bypass
    dst3 = dataclasses.replace(
        dst_slice, ap=[list(dst_slice.ap[0])] + [list(d) for d in dst_dims])
    return eng.add_instruction(
        mybir.InstTensorScalarPtr(
            name=eng.bass.get_next_instruction_name(),
            is_tensor_tensor_scan=True,
            is_scalar_tensor_tensor=True,
            op0=add, op1=byp,
            ins=[eng.lower_ap(src),
                 eng.lower_ap_or_imm(0.0),
                 eng.lower_ap(_sv(zcol, [(0, n)]))],
            outs=[eng.lower_ap(dst3)],
        ))


def build_kernel(split_ctrl=True, debug_taps=False):
    nc = bacc.Bacc("TRN2", target_bir_lowering=False, debug=False)
    add, byp = mybir.AluOpType.add, mybir.AluOpType.bypass
    sub, mul = mybir.AluOpType.subtract, mybir.AluOpType.mult

    x1 = nc.dram_tensor("x1", [2, 128, HW], BF16, kind="ExternalInput")
    xx = nc.dram_tensor("xx", [2, 128, HW], BF16, kind="ExternalInput")
    wqT = nc.dram_tensor("wqT", [2, 128, C], BF16, kind="ExternalInput")
    wkTb = nc.dram_tensor("wkTb", [2, 128, C8], BF16, kind="ExternalInput")
    conz = nc.dram_tensor("conz", [6, 128, 96], BF16, kind="ExternalInput")
    smalls = nc.dram_tensor("smalls", [128, SM_N], F32, kind="ExternalInput")
    out = nc.dram_tensor("out", [2, 128, HW], BF16, kind="ExternalOutput")
    if debug_taps:
        dPq0 = nc.dram_tensor("dPq0", [128, NP], F32, kind="ExternalOutput")
        dPq1 = nc.dram_tensor("dPq1", [128, NP], F32, kind="ExternalOutput")
        dDk = nc.dram_tensor("dDk", [96, NP], F32, kind="ExternalOutput")
        dAff = nc.dram_tensor("dAff", [96, C], F32, kind="ExternalOutput")
        dKq = nc.dram_tensor("dKq", [96, TH * W], F32, kind="ExternalOutput")
        dG0 = nc.dram_tensor("dG0", [128, NB * H], F32, kind="ExternalOutput")

    with tile.TileContext(nc) as tc:
        with (
            tc.tile_pool(name="consts", bufs=1) as cpool,
            tc.tile_pool(name="xpool", bufs=1) as xpool,
            tc.tile_pool(name="scratch", bufs=1) as spool,
            tc.tile_pool(name="ostage", bufs=2) as opool,
        ):
            # ---- tiles ----
            czt = cpool.tile([128, 576], BF16, tag="czt", name="czt")
            wkt = cpool.tile([128, 2 * C8], BF16, tag="wkt", name="wkt")
            wqt = cpool.tile([128, 2 * C], BF16, tag="wqt", name="wqt")
            smt = cpool.tile([128, SM_N], F32, tag="smt", name="smt")
            x1t = [xpool.tile([128, HW], BF16, tag=f"x1t{i}", name=f"x1t{i}") for i in range(2)]
            xxt = [xpool.tile([128, HW], BF16, tag=f"xxt{i}", name=f"xxt{i}") for i in range(2)]

            F1s = [spool.tile([128, HHW + 1], F16, tag=f"F1{i}", name=f"F1{i}") for i in range(3)]
            Gq = [spool.tile([128, NB * H], F16, tag=f"G{i}", name=f"G{i}") for i in range(2)]
            F2q = [spool.tile([128, NB * H + 1], F16, tag=f"F2{i}", name=f"F2{i}") for i in range(2)]
            F1k = spool.tile([96, TH * W + 1], F16, tag="F1k", name="F1k")
            Gk = spool.tile([96, NB * TH], F16, tag="Gk", name="Gk")
            F2k = spool.tile([96, NB * TH + 1], F16, tag="F2k", name="F2k")
            kq = spool.tile([96, TH * W], F16, tag="kq", name="kq")
            zs_raw = spool.tile([96, HW], BF16, tag="zsr", name="zsr")
            zss = spool.tile([96, HW], BF16, tag="zss", name="zss")
            Pq = [spool.tile([128, NP], BF16, tag=f"Pq{i}", name=f"Pq{i}") for i in range(2)]
            Dk = spool.tile([96, NP], F32, tag="Dk", name="Dk")
            Dsh = [spool.tile([32, NP], F32, tag=f"Dsh{t}", name=f"Dsh{t}") for t in range(2)]

            # ---- input DMAs: x1 on SP queue (pooling-critical), xx on Act ----
            Q4 = HHW // 2
            nc.sync.dma_start(czt[:], _sv(conz.ap()[0], [(128 * 96, 6), (1, 96)]))
            nc.sync.dma_start(wkt[:], _sv(wkTb.ap()[0], [(128 * C8, 2), (1, C8)]))
            nc.sync.dma_start(x1t[0][:, :Q4], x1.ap()[0][:, :Q4])
            nc.sync.dma_start(x1t[0][:, Q4:HHW], x1.ap()[0][:, Q4:HHW])
            nc.sync.dma_start(x1t[1][:, :HHW], x1.ap()[1][:, :HHW])
            nc.sync.dma_start(x1t[0][:, HHW:], x1.ap()[0][:, HHW:])
            nc.sync.dma_start(x1t[1][:, HHW:], x1.ap()[1][:, HHW:])
            nc.scalar.dma_start(smt[:], smalls.ap())
            nc.scalar.dma_start(wqt[:], _sv(wqT.ap()[0], [(128 * C, 2), (1, C)]))
            nc.scalar.dma_start(xxt[0][:], xx.ap()[0])
            nc.scalar.dma_start(xxt[1][:], xx.ap()[1])

            bq_t = smt[:, SM_BQ:SM_BQ + C]
            bk_t = smt[:, SM_BK:SM_BK + C8]
            id32 = smt[:, SM_ID:SM_ID + 32]

            # ---- DVE warmup + zero-region init ----
            for t_ in F1s:
                nc.vector.memset(t_[:, 0:1], 0.0)
            nc.vector.memset(F1k[:96, 0:1], 0.0)
            for t_ in F2q:
                nc.vector.memset(t_[:, 0:1], 0.0)
            nc.vector.memset(F2k[:96, 0:1], 0.0)
            nc.gpsimd.memset(Dk[:, :], 0.0)
            nc.gpsimd.memset(zss[0:32, 0:W], 0.0)
            nc.gpsimd.memset(zss[64:96, (H - 1) * W:HW], 0.0)

            # ---- zconv: border-split partial matmuls ----
            def zconv_chunk(pzp, j):
                r0 = j * ROWS
                zp = pzp.tile([96, ROWS * W], F32, tag="zp", name="zp")
                base = r0 * W
                for cinc in range(2):   # dx=1 center, full width, start
                    nc.tensor.matmul(
                        zp[:, :], czt[:, (2 + cinc) * 96:(3 + cinc) * 96],
                        _sv(x1t[cinc][:, base:], [(W, ROWS), (1, W)]),
                        start=(cinc == 0), stop=False)
                for cinc in range(2):   # dx=0 left tap: out 1..95 <- x 0..94
                    nc.tensor.matmul(
                        _sv(zp[:, 1:], [(W, ROWS), (1, W - 1)]),
                        czt[:, (0 + cinc) * 96:(1 + cinc) * 96],
                        _sv(x1t[cinc][:, base:], [(W, ROWS), (1, W - 1)]),
                        start=False, stop=False)
                for cinc in range(2):   # dx=2 right tap: out 0..94 <- x 1..95
                    nc.tensor.matmul(
                        _sv(zp[:, 0:], [(W, ROWS), (1, W - 1)]),
                        czt[:, (4 + cinc) * 96:(5 + cinc) * 96],
                        _sv(x1t[cinc][:, base + 1:], [(W, ROWS), (1, W - 1)]),
                        start=False, stop=(cinc == 1))
                nc.scalar.copy(zs_raw[:, base:base + ROWS * W], zp[:, :])

            with tc.tile_pool(name="pz", bufs=3, space="PSUM") as pzp, \
                 tc.tile_pool(name="pk", bufs=2, space="PSUM") as pkp:
                for j in range(12):
                    zconv_chunk(pzp, j)
                # k-proj off-major: one [96,512] copy per off
                for off in range(6):
                    kp = pkp.tile([128, 512], F32, tag="kp", name="kp")
                    for t in range(3):
                        for cc in range(2):
                            nc.tensor.matmul(kp[32 * t:32 * t + 32, :],
                                             wkt[:, cc * C8:(cc + 1) * C8],
                                             xxt[cc][:, (t * 6 + off) * 512:(t * 6 + off + 1) * 512],
                                             start=(cc == 0), stop=(cc == 1),
                                             tile_position=(0, 32 * t))
                    nc.scalar.copy(kq[0:96, off * 512:(off + 1) * 512], kp[0:96, :])
                for j in range(12, NCH):
                    zconv_chunk(pzp, j)

            # ---- pooling: h-major prefix scans (DVE), diffs split DVE/GPSIMD ----
            add_, byp_ = mybir.AluOpType.add, mybir.AluOpType.bypass

            def scan(F1, src, n):
                nc.vector.tensor_tensor_scan(
                    F1[:, 1:1 + n], src, _sv(F1[:, 0:1], [(0, n)]), 0.0, add_, byp_)

            def w_diffs(eng, F1, G, h0, rows, part=128):
                jbase = 0
                for s_ in POOL_SIZES:
                    sb = _pool_bins(W, s_)
                    for (i0_, cnt, ds, de) in _uniform_runs(sb):
                        ws, we = sb[i0_]
                        j0 = jbase + i0_
                        eng.tensor_tensor(
                            _sv(G[:part, j0 * H + h0:], [(H, cnt), (1, rows)]) if part == 128 else
                            _sv(G[:part, j0 * rows + h0:], [(rows, cnt), (1, rows)]),
                            _sv(F1[:part, we:], [(de, cnt), (W, rows)]),
                            _sv(F1[:part, ws:], [(ds, cnt), (W, rows)]),
                            mybir.AluOpType.subtract)
                    jbase += s_

            def h_diffs(eng, F2, P_out):
                for s_ in POOL_SIZES:
                    jb = JBASE[s_]
                    for (o0, cnt, ds, de) in _uniform_runs(HBINS[s_]):
                        hs, he = HBINS[s_][o0]
                        eng.tensor_tensor(
                            _sv(P_out[:, B280[s_] + o0 * s_:], [(s_, cnt), (1, s_)]),
                            _sv(F2[:, jb * H + he:], [(de, cnt), (H, s_)]),
                            _sv(F2[:, jb * H + hs:], [(ds, cnt), (H, s_)]),
                            mybir.AluOpType.subtract)

            # q scans: strips 0..3 = (cc, strip); F1 buffers rotate a,b,c,a
            FB = [F1s[0], F1s[1], F1s[2], F1s[0]]
            for idx, (cc, strip) in enumerate([(0, 0), (0, 1), (1, 0), (1, 1)]):
                scan(FB[idx], x1t[cc][:, strip * HHW:(strip + 1) * HHW], HHW)
                if idx < 3:   # strip diffs d0,d1,d2 on GPSIMD
                    w_diffs(nc.gpsimd, FB[idx], Gq[cc], strip * SR, SR)
            # kW scan + k W-diffs on DVE (fast path to kF2 -> kHd)
            scan(F1k, kq[:, :], TH * W)
            jbase = 0
            for s_ in POOL_SIZES:
                sb = _pool_bins(W, s_)
                for (i0_, cnt, ds, de) in _uniform_runs(sb):
                    ws, we = sb[i0_]
                    j0 = jbase + i0_
                    nc.vector.tensor_tensor(
                        _sv(Gk[:96, j0 * TH:], [(TH, cnt), (1, TH)]),
                        _sv(F1k[:96, we:], [(de, cnt), (W, TH)]),
                        _sv(F1k[:96, ws:], [(ds, cnt), (W, TH)]),
                        mybir.AluOpType.subtract)
                jbase += s_
            scan(F2k, Gk[:96, :], NB * TH)
            scan(F2q[0], Gq[0][:, :], NB * H)
            # k H-diffs (GPSIMD, after kF2; runs parallel to DVE q1 chain)
            for t in range(3):
                for s_ in POOL_SIZES:
                    jb = JBASE[s_]
                    for o, (hs, he) in enumerate(HBINS[s_]):
                        lhs = min(max(hs - TH * t, 0), TH)
                        lhe = min(max(he - TH * t, 0), TH)
                        if lhe <= lhs:
                            continue
                        nc.gpsimd.tensor_tensor(
                            _sv(Dk[32 * t:32 * t + 32, B280[s_] + o * s_:], [(1, s_)]),
                            _sv(F2k[32 * t:32 * t + 32, jb * TH + lhe:], [(TH, s_)]),
                            _sv(F2k[32 * t:32 * t + 32, jb * TH + lhs:], [(TH, s_)]),
                            mybir.AluOpType.subtract)
            # d3 (DVE), q1 F2; H-diffs: Pq0 on GPSIMD, Pq1 on DVE
            w_diffs(nc.vector, FB[3], Gq[1], SR, SR)
            scan(F2q[1], Gq[1][:, :], NB * H)
            h_diffs(nc.gpsimd, F2q[0], Pq[0])
            h_diffs(nc.vector, F2q[1], Pq[1])

            # partition-shift thirds 1,2 of Dk down to base 0 for matmuls
            # (on SP queue, after zss shifts below in emission but ready later)
            nc.sync.dma_start(zss[0:32, W:HW], zs_raw[0:32, 0:HW - W])
            nc.sync.dma_start(zss[32:64, :], zs_raw[32:64, :])
            nc.sync.dma_start(zss[64:96, 0:HW - W], zs_raw[64:96, W:HW])
            for t in range(2):
                nc.sync.dma_start(Dsh[t][:, :], Dk[32 * (t + 1):32 * (t + 2), :])

            # ---- projections / affinity (bf16 chain) ----
            PCH = [(0, 128), (128, 128), (256, 24)]
            pqT = [cpool.tile([n, C], BF16, tag=f"pqT{i}", name=f"pqT{i}")
                   for i, (_, n) in enumerate(PCH)]
            pkR = [cpool.tile([n, 96], BF16, tag=f"pkR{i}", name=f"pkR{i}")
                   for i, (_, n) in enumerate(PCH)]
            affT96 = cpool.tile([96, C], BF16, tag="affT96", name="affT96")

            with tc.tile_pool(name="psmall", bufs=2, space="PSUM") as pps:
                for i, (p0, n) in enumerate(PCH):
                    aicol = smt[:n, SM_AI3 + 32 * i:SM_AI3 + 32 * i + 1]
                    ps2 = pps.tile([n, C8], F32, tag="ps2", name="ps2")
                    dsrc = [Dk, Dsh[0], Dsh[1]]
                    for t in range(3):
                        nc.tensor.matmul(ps2[:], dsrc[t][0:32, p0:p0 + n],
                                         id32[0:32, :],
                                         start=(t == 0), stop=(t == 2))
                    for g in range(3):
                        nc.vector.scalar_tensor_tensor(
                            pkR[i][:, 32 * g:32 * g + 32], ps2[:], aicol,
                            bk_t[:n, :], mul, add)
                for i, (p0, n) in enumerate(PCH):
                    aicol = smt[:n, SM_AI3 + 32 * i:SM_AI3 + 32 * i + 1]
                    ps = pps.tile([n, C], F32, tag="ps", name="ps")
                    for cc in range(2):
                        nc.tensor.matmul(ps[:], Pq[cc][:, p0:p0 + n],
                                         wqt[:, cc * C:(cc + 1) * C],
                                         start=(cc == 0), stop=(cc == 1))
                    nc.vector.scalar_tensor_tensor(
                        pqT[i][:], ps[:], aicol, bq_t[:n, :], mul, add)
                pa = pps.tile([96, C], F32, tag="pa", name="pa")
                for i in range(3):
                    nc.tensor.matmul(pa[:], pkR[i][:], pqT[i][:],
                                     start=(i == 0), stop=(i == 2))
                nc.scalar.activation(affT96[:], pa[:], mybir.ActivationFunctionType.Sigmoid)

            # PE prewarm: ramp tensor clock before the out burst (deps on
            # late tile Pq[1] so the scheduler runs these in the pre-aff gap)
            with tc.tile_pool(name="pw", bufs=1, space="PSUM") as pwp:
                pwt = pwp.tile([96, C], F32, tag="pwt", name="pwt")
                for _r in range(24):
                    nc.tensor.matmul(pwt[:], czt[:, 0:96], Pq[1][:, 0:256],
                                     start=True, stop=True)

            # ---- out matmuls: 24 chunks x 2 cout halves; GB=6 staging ----
            GB = 6
            with tc.tile_pool(name="po", bufs=4, space="PSUM") as pop:
                ot = [None, None]
                for j in range(NCH):
                    r0 = j * ROWS
                    for coutc in range(2):
                        op = pop.tile([128, ROWS * W], F32, tag="op", name="op")
                        nc.tensor.matmul(op[:], affT96[:, coutc * 128:(coutc + 1) * 128],
                                         zss[:, r0 * W:(r0 + ROWS) * W],
                                         start=True, stop=True)
                        if j % GB == 0:
                            ot[coutc] = opool.tile([128, GB * ROWS * W], BF16,
                                                   tag=f"ot{coutc}", name=f"ot{coutc}")
                        seg = (j % GB) * ROWS * W
                        if (j + coutc) % 2 == 0:
                            nc.scalar.copy(ot[coutc][:, seg:seg + ROWS * W], op[:])
                        else:
                            nc.vector.tensor_copy(ot[coutc][:, seg:seg + ROWS * W], op[:])
                        if j % GB == GB - 1:
                            nc.sync.dma_start(
                                out.ap()[coutc][:, (r0 - (GB - 1) * ROWS) * W:(r0 + ROWS) * W],
                                ot[coutc][:])

            if debug_taps:
                dbg = cpool.tile([128, NP], F32, tag="dbg", name="dbg")
                dbg2 = cpool.tile([128, TH * W], F32, tag="dbg2", name="dbg2")
                nc.vector.tensor_copy(dbg[:, :], Pq[0][:, :])
                nc.sync.dma_start(dPq0.ap(), dbg[:, :])
                nc.vector.tensor_copy(dbg[:, :], Pq[1][:, :])
                nc.sync.dma_start(dPq1.ap(), dbg[:, :])
                nc.sync.dma_start(dDk.ap(), Dk[:, :])
                nc.vector.tensor_copy(dbg[0:96, 0:C], affT96[:, :])
                nc.sync.dma_start(dAff.ap(), dbg[0:96, 0:C])
                nc.vector.tensor_copy(dbg2[0:96, 0:TH * W], kq[:, :])
                nc.sync.dma_start(dKq.ap(), dbg2[0:96, 0:TH * W])
                nc.vector.tensor_copy(dbg2[:, 0:NB * H], Gq[0][:, :])
                nc.sync.dma_start(dG0.ap(), dbg2[:, 0:NB * H])

    if split_ctrl:
        nc.compile()
        _fix_ldweights_waits(nc)
    return nc


_NC_CACHE = {}


def _get_nc():
    if "nc" not in _NC_CACHE:
        _NC_CACHE["nc"] = build_kernel()
    return _NC_CACHE["nc"]


def _conv_cast(x):
    import ml_dtypes
    return np.ascontiguousarray(x, np.float32).astype(ml_dtypes.bfloat16)


def kernel(x_1, x, wq, bq, wk, bk, con):
    import ml_dtypes
    x_1 = _conv_cast(x_1)
    x = _conv_cast(x)
    con = np.asarray(con, np.float32)
    wq = np.asarray(wq, np.float32)
    bq = np.asarray(bq, np.float32)
    wk = np.asarray(wk, np.float32)
    bk = np.asarray(bk, np.float32)

    wqT_h = np.ascontiguousarray(wq.T).reshape(2, 128, C).astype(ml_dtypes.bfloat16)
    wkTb_h = np.ascontiguousarray(wk.T).reshape(2, 128, C8).astype(ml_dtypes.bfloat16)
    conz_h = np.ascontiguousarray(
        con.transpose(3, 1, 2, 0)          # [dx, cin256, dy, kk]
        .reshape(3, 2, 128, 3 * C8)
        .reshape(6, 128, 96)
    ).astype(ml_dtypes.bfloat16)
    ai = _area_inv()
    smalls_h = np.zeros((128, SM_N), np.float32)
    smalls_h[:, SM_BQ:SM_BQ + C] = bq[None, :]
    smalls_h[:, SM_BK:SM_BK + C8] = bk[None, :]
    smalls_h[:, SM_AI:SM_AI + NP] = ai[None, :]
    for i, (p0, n) in enumerate([(0, 128), (128, 128), (256, 24)]):
        smalls_h[:n, SM_AI3 + 32 * i:SM_AI3 + 32 * (i + 1)] = ai[p0:p0 + n, None]
    smalls_h[:32, SM_ID:SM_ID + 32] = np.eye(32, dtype=np.float32)

    in_maps = []
    for b in range(B):
        in_maps.append({
            "x1": x_1[b].reshape(2, 128, HW),
            "xx": x[b].reshape(2, 128, HW),
            "wqT": wqT_h, "wkTb": wkTb_h, "conz": conz_h, "smalls": smalls_h,
        })
    global _last_in_maps
    _last_in_maps = in_maps
    nc = _get_nc()
    res = run_bass_kernel_spmd(nc, in_maps, list(range(B)))
    return np.stack([res.results[b]["out"].astype(np.float32).reshape(C, H, W)
                     for b in range(B)])


# revision 20
# speedup vs baseline: 1.1531x; 1.0895x over previous
"""Trainium2 Bass kernel for nn_ASSC_66657892434080 (v3).

Reference computation (per batch sample b, data-parallel over 8 cores):
    q = wq @ x_1[b] + bq ; k = wk @ x[b] + bk          (1x1 convs)
    proj_query = PSP(q) [256,280] ; proj_key = PSP(k) [32,280]
    aff = sigmoid(proj_query @ proj_key^T)             [256,32]
    out[b] = conv3x3(x_1[b], (aff @ con.reshape(32,-1)))   (grouped conv)

v3 key mechanisms (see kernel_v1/v2 baks for history):
  * contiguous [128, 9216] x loads split over BOTH HWDGE queues (SP + Act).
  * conv borders via partial-extent matmuls; dy shift via 3 SBUF->SBUF DMAs.
  * prefix scans (DVE-only) write TRANSPOSED dsts via hand-built 3-dim APs
    (validated bit-exact on HW): F1T[w,h] so every bin-diff has a contiguous
    inner dim -> diffs run fast on GPSIMD; same trick for the H-stage (F2T).
  * area-normalization folded into scalar_tensor_tensor bias-adds.
  * affinity matmul chain in bf16.
"""

import numpy as np
import dataclasses
import concourse.bass as bass
import concourse.bacc as bacc
import concourse.tile as tile
import concourse.mybir as mybir
import bass_rust
from concourse.bass_utils import run_bass_kernel_spmd

B, C, H, W = 8, 256, 96, 96
C8 = 32
HW = H * W                      # 9216
POOL_SIZES = (1, 3, 5, 7, 14)   # -> 30 1-D bins, 280 2-D positions
NB = sum(POOL_SIZES)            # 30
NP = sum(s * s for s in POOL_SIZES)  # 280
TH = 32                         # k-side rows per third
ROWS = 4                        # conv rows per PSUM chunk
NCH = H // ROWS                 # 24 chunks
HHW = HW // 2                   # 4608
SR = 48                         # strip rows (q side)
F32 = mybir.dt.float32
BF16 = mybir.dt.bfloat16
F16 = mybir.dt.float16

# smalls packing (f32, [128, 696]): bq | bk | ai | ai3(3x32) | id32
SM_BQ, SM_BK, SM_AI, SM_AI3, SM_ID, SM_N = 0, 256, 288, 568, 664, 696


def _pool_bins(n, s):
    return [((i * n) // s, -((-(i + 1) * n) // s)) for i in range(s)]


HBINS = {s: _pool_bins(H, s) for s in POOL_SIZES}
JBASE = {}
B280 = {}
_j = _p = 0
for _s in POOL_SIZES:
    JBASE[_s] = _j
    B280[_s] = _p
    _j += _s
    _p += _s * _s


def _area_inv():
    ai = np.zeros(NP, np.float32)
    for s in POOL_SIZES:
        hb, wb = _pool_bins(H, s), _pool_bins(W, s)
        for o, (hs, he) in enumerate(hb):
            for p, (ws, we) in enumerate(wb):
                ai[B280[s] + o * s + p] = 1.0 / ((he - hs) * (we - ws))
    return ai


def _split_multiwait_ctrl(nc, default_limit=1):
    """walrus in this container rejects instructions carrying more than one
    sem wait; move extras onto preceding same-engine drains.  NEVER split PE
    instructions (reorder window pulls LDWEIGHTS ahead)."""
    for f in nc.m.functions:
        for bb in f.blocks:
            new_list = []
            for inst in bb.instructions:
                si = inst.sync_info
                waits = list(si.on_wait) if si and si.on_wait else []
                mw = default_limit
                if getattr(inst, "engine", None) == mybir.EngineType.PE:
                    mw = 99
                if len(waits) > mw:
                    for k, w in enumerate(waits[:-mw]):
                        pre = mybir.InstDrain(name=f"{inst.name}-w{k}", ins=[], outs=[])
                        pre.engine = inst.engine
                        pre.sync_info = bass_rust.SyncInfo(on_wait=[w], on_update=[])
                        new_list.append(pre)
                    inst.sync_info = bass_rust.SyncInfo(
                        on_wait=waits[-mw:],
                        on_update=list(si.on_update) if si.on_update else [],
                    )
                new_list.append(inst)
            bb.instructions[:] = new_list


def _sv(ap2d, dims):
    """Strided view: keep partition dim, replace free dims with (step, count)."""
    return dataclasses.replace(ap2d, ap=[list(ap2d.ap[0])] + [[s, c] for s, c in dims])


def _uniform_runs(bins):
    """Group consecutive bins into runs with constant boundary strides."""
    runs = []
    i = 0
    n = len(bins)
    while i < n:
        if i == n - 1:
            runs.append((i, 1, 0, 0))
            i += 1
            continue
        ds = bins[i + 1][0] - bins[i][0]
        de = bins[i + 1][1] - bins[i][1]
        j = i + 1
        while (j + 1 < n and bins[j + 1][0] - bins[j][0] == ds
               and bins[j + 1][1] - bins[j][1] == de):
            j += 1
        runs.append((i, j - i + 1, ds, de))
        i = j + 1
    return runs


def _fix_ldweights_waits(nc):
    """Move waits that gate weight data from InstMatmult to its InstLdweights
    (prevents stale-weight races after Tile's 2-byte matmul split)."""
    import copy
    for f in nc.m.functions:
        for bb in f.blocks:
            insts = bb.instructions
            new_list = []
            i = 0
            while i < len(insts):
                inst = insts[i]
                nxt = insts[i + 1] if i + 1 < len(insts) else None
                if (type(inst).__name__ == "InstLdweights" and nxt is not None
                        and type(nxt).__name__ == "InstMatmult"):
                    wl = list(inst.sync_info.on_wait) if inst.sync_info and inst.sync_info.on_wait else []
                    wm = list(nxt.sync_info.on_wait) if nxt.sync_info and nxt.sync_info.on_wait else []
                    waits = wl + wm
                    mm_upd = list(nxt.sync_info.on_update) if nxt.sync_info and nxt.sync_info.on_update else []
                    ld_upd = list(inst.sync_info.on_update) if inst.sync_info and inst.sync_info.on_update else []
                    if len(waits) > 1:
                        for k, w in enumerate(waits[:-1]):
                            pre = copy.deepcopy(inst)
                            pre.name = f"{inst.name}-ldw{k}"
                            pre.sync_info = bass_rust.SyncInfo(on_wait=[w], on_update=[])
                            new_list.append(pre)
                        inst.sync_info = bass_rust.SyncInfo(on_wait=[waits[-1]], on_update=ld_upd)
                        nxt.sync_info = bass_rust.SyncInfo(on_wait=[], on_update=mm_upd)
                    elif len(waits) == 1:
                        inst.sync_info = bass_rust.SyncInfo(on_wait=[waits[0]], on_update=ld_upd)
                        nxt.sync_info = bass_rust.SyncInfo(on_wait=[], on_update=mm_upd)
                    new_list.append(inst)
                    new_list.append(nxt)
                    i += 2
                    continue
                new_list.append(inst)
                i += 1
            bb.instructions[:] = new_list


def _scan3(nc, dst_slice, dst_dims, src, zcol, n):
    """tensor_tensor_scan with a hand-built multi-dim (e.g. transposed) dst AP.
    Bypasses the 2-dim assert in bass; validated bit-exact on HW.  `zcol` is a
    [*, 1] column whose (bypassed) read also serves as an ordering token."""
    eng = nc.vector
    add, byp = mybir.AluOpType.add, mybir.AluOpType.bypass
    dst3 = dataclasses.replace(
        dst_slice, ap=[list(dst_slice.ap[0])] + [list(d) for d in dst_dims])
    return eng.add_instruction(
        mybir.InstTensorScalarPtr(
            name=eng.bass.get_next_instruction_name(),
            is_tensor_tensor_scan=True,
            is_scalar_tensor_tensor=True,
            op0=add, op1=byp,
            ins=[eng.lower_ap(src),
                 eng.lower_ap_or_imm(0.0),
                 eng.lower_ap(_sv(zcol, [(0, n)]))],
            outs=[eng.lower_ap(dst3)],
        ))


def build_kernel(split_ctrl=True, debug_taps=False):
    nc = bacc.Bacc("TRN2", target_bir_lowering=False, debug=False)
    add, byp = mybir.AluOpType.add, mybir.AluOpType.bypass
    sub, mul = mybir.AluOpType.subtract, mybir.AluOpType.mult

    x1 = nc.dram_tensor("x1", [2, 128, HW], BF16, kind="ExternalInput")
    xx = nc.dram_tensor("xx", [2, 128, HW], BF16, kind="ExternalInput")
    wqT = nc.dram_tensor("wqT", [2, 128, C], BF16, kind="ExternalInput")
    wkTb = nc.dram_tensor("wkTb", [2, 128, C8], BF16, kind="ExternalInput")
    conz = nc.dram_tensor("conz", [6, 128, 96], BF16, kind="ExternalInput")
    smalls = nc.dram_tensor("smalls", [128, SM_N], F32, kind="ExternalInput")
    out = nc.dram_tensor("out", [2, 128, HW], BF16, kind="ExternalOutput")
    if debug_taps:
        dPq0 = nc.dram_tensor("dPq0", [128, NP], F32, kind="ExternalOutput")
        dPq1 = nc.dram_tensor("dPq1", [128, NP], F32, kind="ExternalOutput")
        dDk = nc.dram_tensor("dDk", [96, NP], F32, kind="ExternalOutput")
        dAff = nc.dram_tensor("dAff", [96, C], F32, kind="ExternalOutput")
        dKq = nc.dram_tensor("dKq", [96, TH * W], F32, kind="ExternalOutput")
        dG0 = nc.dram_tensor("dG0", [128, NB * H], F32, kind="ExternalOutput")

    with tile.TileContext(nc) as tc:
        with (
            tc.tile_pool(name="consts", bufs=1) as cpool,
            tc.tile_pool(name="xpool", bufs=1) as xpool,
            tc.tile_pool(name="scratch", bufs=1) as spool,
            tc.tile_pool(name="ostage", bufs=2) as opool,
        ):
            # ---- tiles ----
            czt = cpool.tile([128, 576], BF16, tag="czt", name="czt")
            wkt = cpool.tile([128, 2 * C8], BF16, tag="wkt", name="wkt")
            wqt = cpool.tile([128, 2 * C], BF16, tag="wqt", name="wqt")
            smt = cpool.tile([128, SM_N], F32, tag="smt", name="smt")
            x1t = [xpool.tile([128, HW], BF16, tag=f"x1t{i}", name=f"x1t{i}") for i in range(2)]
            xxt = [xpool.tile([128, HW], BF16, tag=f"xxt{i}", name=f"xxt{i}") for i in range(2)]

            F1s = [spool.tile([128, HHW + 1], F16, tag=f"F1{i}", name=f"F1{i}") for i in range(3)]
            Gq = [spool.tile([128, NB * H], F16, tag=f"G{i}", name=f"G{i}") for i in range(2)]
            F2q = [spool.tile([128, NB * H + 1], F16, tag=f"F2{i}", name=f"F2{i}") for i in range(2)]
            F1k = spool.tile([96, TH * W + 1], F16, tag="F1k", name="F1k")
            Gk = spool.tile([96, NB * TH], F16, tag="Gk", name="Gk")
            F2k = spool.tile([96, NB * TH + 1], F16, tag="F2k", name="F2k")
            kq = spool.tile([96, TH * W], F16, tag="kq", name="kq")
            zs_raw = spool.tile([96, HW], BF16, tag="zsr", name="zsr")
            zss = spool.tile([96, HW], BF16, tag="zss", name="zss")
            Pq = [spool.tile([128, NP], BF16, tag=f"Pq{i}", name=f"Pq{i}") for i in range(2)]
            Dk = spool.tile([96, NP], F32, tag="Dk", name="Dk")
            Dsh = [spool.tile([32, NP], F32, tag=f"Dsh{t}", name=f"Dsh{t}") for t in range(2)]

            # ---- input DMAs: x1 on SP queue (pooling-critical), xx on Act ----
            Q4 = HHW // 2
            nc.sync.dma_start(x1t[0][:, :Q4], x1.ap()[0][:, :Q4])
            nc.sync.dma_start(x1t[0][:, Q4:HHW], x1.ap()[0][:, Q4:HHW])
            nc.sync.dma_start(x1t[0][:, HHW:], x1.ap()[0][:, HHW:])
            nc.sync.dma_start(x1t[1][:, :HHW], x1.ap()[1][:, :HHW])
            nc.sync.dma_start(x1t[1][:, HHW:], x1.ap()[1][:, HHW:])
            nc.sync.dma_start(czt[:], _sv(conz.ap()[0], [(128 * 96, 6), (1, 96)]))
            nc.sync.dma_start(wkt[:], _sv(wkTb.ap()[0], [(128 * C8, 2), (1, C8)]))
            nc.scalar.dma_start(smt[:], smalls.ap())
            nc.scalar.dma_start(wqt[:], _sv(wqT.ap()[0], [(128 * C, 2), (1, C)]))
            nc.scalar.dma_start(xxt[1][:], xx.ap()[1])
            nc.scalar.dma_start(xxt[0][:], xx.ap()[0])

            bq_t = smt[:, SM_BQ:SM_BQ + C]
            bk_t = smt[:, SM_BK:SM_BK + C8]
            id32 = smt[:, SM_ID:SM_ID + 32]

            # ---- DVE warmup + zero-region init ----
            for t_ in F1s:
                nc.vector.memset(t_[:, 0:1], 0.0)
            nc.vector.memset(F1k[:96, 0:1], 0.0)
            for t_ in F2q:
                nc.vector.memset(t_[:, 0:1], 0.0)
            nc.vector.memset(F2k[:96, 0:1], 0.0)
            nc.gpsimd.memset(Dk[:, :], 0.0)
            nc.gpsimd.memset(zss[0:32, 0:W], 0.0)
            nc.gpsimd.memset(zss[64:96, (H - 1) * W:HW], 0.0)

            # ---- zconv: border-split partial matmuls ----
            def zconv_chunk(pzp, j):
                r0 = j * ROWS
                zp = pzp.tile([96, ROWS * W], F32, tag="zp", name="zp")
                base = r0 * W
                for cinc in range(2):   # dx=1 center, full width, start
                    nc.tensor.matmul(
                        zp[:, :], czt[:, (2 + cinc) * 96:(3 + cinc) * 96],
                        _sv(x1t[cinc][:, base:], [(W, ROWS), (1, W)]),
                        start=(cinc == 0), stop=False)
                for cinc in range(2):   # dx=0 left tap: out 1..95 <- x 0..94
                    nc.tensor.matmul(
                        _sv(zp[:, 1:], [(W, ROWS), (1, W - 1)]),
                        czt[:, (0 + cinc) * 96:(1 + cinc) * 96],
                        _sv(x1t[cinc][:, base:], [(W, ROWS), (1, W - 1)]),
                        start=False, stop=False)
                for cinc in range(2):   # dx=2 right tap: out 0..94 <- x 1..95
                    nc.tensor.matmul(
                        _sv(zp[:, 0:], [(W, ROWS), (1, W - 1)]),
                        czt[:, (4 + cinc) * 96:(5 + cinc) * 96],
                        _sv(x1t[cinc][:, base + 1:], [(W, ROWS), (1, W - 1)]),
                        start=False, stop=(cinc == 1))
                nc.scalar.copy(zs_raw[:, base:base + ROWS * W], zp[:, :])

            with tc.tile_pool(name="pz", bufs=3, space="PSUM") as pzp, \
                 tc.tile_pool(name="pk", bufs=2, space="PSUM") as pkp:
                for j in range(12):
                    zconv_chunk(pzp, j)
                # k-proj off-major: one [96,512] copy per off
                for off in range(6):
                    kp = pkp.tile([128, 512], F32, tag="kp", name="kp")
                    for t in range(3):
                        for cc in range(2):
                            nc.tensor.matmul(kp[32 * t:32 * t + 32, :],
                                             wkt[:, cc * C8:(cc + 1) * C8],
                                             xxt[cc][:, (t * 6 + off) * 512:(t * 6 + off + 1) * 512],
                                             start=(cc == 0), stop=(cc == 1),
                                             tile_position=(0, 32 * t))
                    nc.scalar.copy(kq[0:96, off * 512:(off + 1) * 512], kp[0:96, :])
                for j in range(12, NCH):
                    zconv_chunk(pzp, j)

            # ---- pooling: h-major prefix scans (DVE), diffs split DVE/GPSIMD ----
            add_, byp_ = mybir.AluOpType.add, mybir.AluOpType.bypass

            def scan(F1, src, n):
                nc.vector.tensor_tensor_scan(
                    F1[:, 1:1 + n], src, _sv(F1[:, 0:1], [(0, n)]), 0.0, add_, byp_)

            def w_diffs(eng, F1, G, h0, rows, part=128):
                jbase = 0
                for s_ in POOL_SIZES:
                    sb = _pool_bins(W, s_)
                    for (i0_, cnt, ds, de) in _uniform_runs(sb):
                        ws, we = sb[i0_]
                        j0 = jbase + i0_
                        eng.tensor_tensor(
                            _sv(G[:part, j0 * H + h0:], [(H, cnt), (1, rows)]) if part == 128 else
                            _sv(G[:part, j0 * rows + h0:], [(rows, cnt), (1, rows)]),
                            _sv(F1[:part, we:], [(de, cnt), (W, rows)]),
                            _sv(F1[:part, ws:], [(ds, cnt), (W, rows)]),
                            mybir.AluOpType.subtract)
                    jbase += s_

            def h_diffs(eng, F2, P_out):
                for s_ in POOL_SIZES:
                    jb = JBASE[s_]
                    for (o0, cnt, ds, de) in _uniform_runs(HBINS[s_]):
                        hs, he = HBINS[s_][o0]
                        eng.tensor_tensor(
                            _sv(P_out[:, B280[s_] + o0 * s_:], [(s_, cnt), (1, s_)]),
                            _sv(F2[:, jb * H + he:], [(de, cnt), (H, s_)]),
                            _sv(F2[:, jb * H + hs:], [(ds, cnt), (H, s_)]),
                            mybir.AluOpType.subtract)

            # q scans: strips 0..3 = (cc, strip); F1 buffers rotate a,b,c,a
            FB = [F1s[0], F1s[1], F1s[2], F1s[0]]
            for idx, (cc, strip) in enumerate([(0, 0), (0, 1), (1, 0), (1, 1)]):
                scan(FB[idx], x1t[cc][:, strip * HHW:(strip + 1) * HHW], HHW)
                if idx < 2:   # chunk-0 W-diffs on GPSIMD
                    w_diffs(nc.gpsimd, FB[idx], Gq[cc], strip * SR, SR)
            scan(F1k, kq[:, :], TH * W)
            # k W-diffs on GPSIMD
            jbase = 0
            for s_ in POOL_SIZES:
                sb = _pool_bins(W, s_)
                for (i0_, cnt, ds, de) in _uniform_runs(sb):
                    ws, we = sb[i0_]
                    j0 = jbase + i0_
                    nc.gpsimd.tensor_tensor(
                        _sv(Gk[:96, j0 * TH:], [(TH, cnt), (1, TH)]),
                        _sv(F1k[:96, we:], [(de, cnt), (W, TH)]),
                        _sv(F1k[:96, ws:], [(ds, cnt), (W, TH)]),
                        mybir.AluOpType.subtract)
                jbase += s_
            # chunk-1 W-diffs on DVE, then F2 scans
            w_diffs(nc.vector, FB[2], Gq[1], 0, SR)
            w_diffs(nc.vector, FB[3], Gq[1], SR, SR)
            scan(F2k, Gk[:96, :], NB * TH)
            scan(F2q[0], Gq[0][:, :], NB * H)
            # k H-diffs (GPSIMD, parallel to DVE q1 chain)
            for t in range(3):
                for s_ in POOL_SIZES:
                    jb = JBASE[s_]
                    for o, (hs, he) in enumerate(HBINS[s_]):
                        lhs = min(max(hs - TH * t, 0), TH)
                        lhe = min(max(he - TH * t, 0), TH)
                        if lhe <= lhs:
                            continue
                        nc.gpsimd.tensor_tensor(
                            _sv(Dk[32 * t:32 * t + 32, B280[s_] + o * s_:], [(1, s_)]),
                            _sv(F2k[32 * t:32 * t + 32, jb * TH + lhe:], [(TH, s_)]),
                            _sv(F2k[32 * t:32 * t + 32, jb * TH + lhs:], [(TH, s_)]),
                            mybir.AluOpType.subtract)
            scan(F2q[1], Gq[1][:, :], NB * H)
            h_diffs(nc.vector, F2q[0], Pq[0])
            h_diffs(nc.vector, F2q[1], Pq[1])

            # partition-shift thirds 1,2 of Dk down to base 0 for matmuls
            # (on SP queue, after zss shifts below in emission but ready later)
            nc.sync.dma_start(zss[0:32, W:HW], zs_raw[0:32, 0:HW - W])
            nc.sync.dma_start(zss[32:64, :], zs_raw[32:64, :])
            nc.sync.dma_start(zss[64:96, 0:HW - W], zs_raw[64:96, W:HW])
            for t in range(2):
                nc.sync.dma_start(Dsh[t][:, :], Dk[32 * (t + 1):32 * (t + 2), :])

            # ---- projections / affinity (bf16 chain) ----
            PCH = [(0, 128), (128, 128), (256, 24)]
            pqT = [cpool.tile([n, C], BF16, tag=f"pqT{i}", name=f"pqT{i}")
                   for i, (_, n) in enumerate(PCH)]
            pkR = [cpool.tile([n, 96], BF16, tag=f"pkR{i}", name=f"pkR{i}")
                   for i, (_, n) in enumerate(PCH)]
            affT96 = cpool.tile([96, C], BF16, tag="affT96", name="affT96")

            with tc.tile_pool(name="psmall", bufs=2, space="PSUM") as pps:
                for i, (p0, n) in enumerate(PCH):
                    aicol = smt[:n, SM_AI3 + 32 * i:SM_AI3 + 32 * i + 1]
                    ps2 = pps.tile([n, C8], F32, tag="ps2", name="ps2")
                    dsrc = [Dk, Dsh[0], Dsh[1]]
                    for t in range(3):
                        nc.tensor.matmul(ps2[:], dsrc[t][0:32, p0:p0 + n],
                                         id32[0:32, :],
                                         start=(t == 0), stop=(t == 2))
                    for g in range(3):
                        nc.vector.scalar_tensor_tensor(
                            pkR[i][:, 32 * g:32 * g + 32], ps2[:], aicol,
                            bk_t[:n, :], mul, add)
                for i, (p0, n) in enumerate(PCH):
                    aicol = smt[:n, SM_AI3 + 32 * i:SM_AI3 + 32 * i + 1]
                    ps = pps.tile([n, C], F32, tag="ps", name="ps")
                    for cc in range(2):
                        nc.tensor.matmul(ps[:], Pq[cc][:, p0:p0 + n],
                                         wqt[:, cc * C:(cc + 1) * C],
                                         start=(cc == 0), stop=(cc == 1))
                    nc.vector.scalar_tensor_tensor(
                        pqT[i][:], ps[:], aicol, bq_t[:n, :], mul, add)
                pa = pps.tile([96, C], F32, tag="pa", name="pa")
                for i in range(3):
                    nc.tensor.matmul(pa[:], pkR[i][:], pqT[i][:],
                                     start=(i == 0), stop=(i == 2))
                nc.scalar.activation(affT96[:], pa[:], mybir.ActivationFunctionType.Sigmoid)

            # PE prewarm: ramp tensor clock before the out burst (deps on
            # late tile Pq[1] so the scheduler runs these in the pre-aff gap)
            with tc.tile_pool(name="pw", bufs=1, space="PSUM") as pwp:
                pwt = pwp.tile([96, C], F32, tag="pwt", name="pwt")
                for _r in range(24):
                    nc.tensor.matmul(pwt[:], czt[:, 0:96], Pq[1][:, 0:256],
                                     start=True, stop=True)

            # ---- out matmuls: 24 chunks x 2 cout halves; GB=6 staging ----
            GB = 6
            with tc.tile_pool(name="po", bufs=4, space="PSUM") as pop:
                ot = [None, None]
                for j in range(NCH):
                    r0 = j * ROWS
                    for coutc in range(2):
                        op = pop.tile([128, ROWS * W], F32, tag="op", name="op")
                        nc.tensor.matmul(op[:], affT96[:, coutc * 128:(coutc + 1) * 128],
                                         zss[:, r0 * W:(r0 + ROWS) * W],
                                         start=True, stop=True)
                        if j % GB == 0:
                            ot[coutc] = opool.tile([128, GB * ROWS * W], BF16,
                                                   tag=f"ot{coutc}", name=f"ot{coutc}")
                        seg = (j % GB) * ROWS * W
                        if (j + coutc) % 2 == 0:
                            nc.scalar.copy(ot[coutc][:, seg:seg + ROWS * W], op[:])
                        else:
                            nc.vector.tensor_copy(ot[coutc][:, seg:seg + ROWS * W], op[:])
                        if j % GB == GB - 1:
                            nc.sync.dma_start(
                                out.ap()[coutc][:, (r0 - (GB - 1) * ROWS) * W:(r0 + ROWS) * W],
                                ot[coutc][:])

            if debug_taps:
                dbg = cpool.tile([128, NP], F32, tag="dbg", name="dbg")
                dbg2 = cpool.tile([128, TH * W], F32, tag="dbg2", name="dbg2")
                nc.vector.tensor_copy(dbg[:, :], Pq[0][:, :])
                nc.sync.dma_start(dPq0.ap(), dbg[:, :])
                nc.vector.tensor_copy(dbg[:, :], Pq[1][:, :])
                nc.sync.dma_start(dPq1.ap(), dbg[:, :])
                nc.sync.dma_start(dDk.ap(), Dk[:, :])
                nc.vector.tensor_copy(dbg[0:96, 0:C], affT96[:, :])
                nc.sync.dma_start(dAff.ap(), dbg[0:96, 0:C])
                nc.vector.tensor_copy(dbg2[0:96, 0:TH * W], kq[:, :])
                nc.sync.dma_start(dKq.ap(), dbg2[0:96, 0:TH * W])
                nc.vector.tensor_copy(dbg2[:, 0:NB * H], Gq[0][:, :])
                nc.sync.dma_start(dG0.ap(), dbg2[:, 0:NB * H])

    if split_ctrl:
        nc.compile()
        _fix_ldweights_waits(nc)
    return nc


_NC_CACHE = {}


def _get_nc():
    if "nc" not in _NC_CACHE:
        _NC_CACHE["nc"] = build_kernel()
    return _NC_CACHE["nc"]


def _conv_cast(x):
    import ml_dtypes
    return np.ascontiguousarray(x, np.float32).astype(ml_dtypes.bfloat16)


def kernel(x_1, x, wq, bq, wk, bk, con):
    import ml_dtypes
    x_1 = _conv_cast(x_1)
    x = _conv_cast(x)
    con = np.asarray(con, np.float32)
    wq = np.asarray(wq, np.float32)
    bq = np.asarray(bq, np.float32)
    wk = np.asarray(wk, np.float32)
    bk = np.asarray(bk, np.float32)

    wqT_h = np.ascontiguousarray(wq.T).reshape(2, 128, C).astype(ml_dtypes.bfloat16)
    wkTb_h = np.ascontiguousarray(wk.T).reshape(2, 128, C8).astype(ml_dtypes.bfloat16)
    conz_h = np.ascontiguousarray(
        con.transpose(3, 1, 2, 0)          # [dx, cin256, dy, kk]
        .reshape(3, 2, 128, 3 * C8)
        .reshape(6, 128, 96)
    ).astype(ml_dtypes.bfloat16)
    ai = _area_inv()
    smalls_h = np.zeros((128, SM_N), np.float32)
    smalls_h[:, SM_BQ:SM_BQ + C] = bq[None, :]
    smalls_h[:, SM_BK:SM_BK + C8] = bk[None, :]
    smalls_h[:, SM_AI:SM_AI + NP] = ai[None, :]
    for i, (p0, n) in enumerate([(0, 128), (128, 128), (256, 24)]):
        smalls_h[:n, SM_AI3 + 32 * i:SM_AI3 + 32 * (i + 1)] = ai[p0:p0 + n, None]
    smalls_h[:32, SM_ID:SM_ID + 32] = np.eye(32, dtype=np.float32)

    in_maps = []
    for b in range(B):
        in_maps.append({
            "x1": x_1[b].reshape(2, 128, HW),
            "xx": x[b].reshape(2, 128, HW),
            "wqT": wqT_h, "wkTb": wkTb_h, "conz": conz_h, "smalls": smalls_h,
        })
    global _last_in_maps
    _last_in_maps = in_maps
    nc = _get_nc()
    res = run_bass_kernel_spmd(nc, in_maps, list(range(B)))
    return np.stack([res.results[b]["out"].astype(np.float32).reshape(C, H, W)
                     for b in range(B)])
